# revision 1
# baseline (speedup 1.0000x reference)
"""Trainium2 Bass kernel for nn_BlockSelfAttention (attention over 8 heads per token).

Math per token t (32768 tokens total, 1024 features = 8 heads x 128 dims):
  xh = x[t].reshape(8, 128)                     # (h, d)
  q = xh @ Wq.T + bq ; k = xh @ Wk.T + bk ; v = xh @ Wv.T + bv
  scores = (q @ k.T) / sqrt(128)                # (8, 8) attention over heads
  out[t] = softmax(scores, -1) @ v              # -> reshape back to 1024

Identities used:
  * bk drops out (adds a per-row constant to scores -> softmax invariant).
  * 1/sqrt(d) and bq are folded into Wq/bq on the host.
  * bv is added to v rows; since softmax rows sum to 1 the output gets +bv.

Layout strategy (per core: 4096 tokens = 32 tiles of 128 tokens):
  * the host pre-transposes/interleaves x into xt[d, t*8+h] so each SBUF
    tile XT [d=128, 1024] holds 128 tokens with every 128-column block
    covering 16 whole tokens (all 8 heads).
  * q/k projections: matmul(lhsT=WqT [d,e], rhs=XT) -> qT2/kT2 [e, (t*8+h)].
  * v projection: matmul(lhsT=XT block [(d),(t,g)], rhs=WvT) ->
    V [(t,g), e] (natural orientation) per 16-token group.
  * scores (transposed): matmul(lhsT=kT2 block, rhs=qT2 block) ->
    S [(t,g), (t,h)] 128x128 per 16-token group; a rank-17 mask matmul
    pre-accumulates -30000 on off-token-diagonal entries (so exp -> 0).
  * exp on scalar engine (no max subtraction: |scores| < ~10).
  * denominator: matmul(lhsT=P, rhs=ones) -> [(t,h), 1], batched 8/bank.
  * AV: matmul(lhsT=P, rhs=V) -> out [(t,h), e]; multiply by reciprocal
    denominator (per-partition broadcast) and DMA out; the (t,h) x e tile
    maps to contiguous DRAM rows.
"""

import numpy as np

HEADS = 8
D = 128
B, N, F = 8, 4096, 1024
NCORES = 8
TOK = (B * N) // NCORES          # tokens per core
P = 128                          # tokens per tile
NT = TOK // P                    # tiles per core
NEG = -30000.0

_NC_CACHE = {}


def _build_nc(mm_dt_name="f32", BUFS=None, reps=1):
    import concourse.mybir as mybir
    import concourse.tile as tile
    from concourse import bacc
    from contextlib import ExitStack

    f32 = mybir.dt.float32
    bf16 = mybir.dt.bfloat16
    if mm_dt_name in ("f32", "f32r"):
        mm_dt = f32
    elif mm_dt_name == "bf16":
        mm_dt = bf16
    else:
        raise ValueError(mm_dt_name)

    def mm(ap):
        # reinterpret an fp32 AP as fp32r at matmul call sites
        if mm_dt_name == "f32r" and ap.dtype == f32:
            return ap.bitcast(mybir.dt.float32r)
        return ap

    BUFS = BUFS or {}
    SCR_OUT = bool(BUFS.get("scr_out", 1))
    nc = bacc.Bacc("TRN2", target_bir_lowering=False, debug=False)

    xt = nc.dram_tensor("xt", [D, TOK * HEADS], mm_dt, kind="ExternalInput")
    zmt = nc.dram_tensor("zmt", [D, D], mm_dt, kind="ExternalInput")
    wvt = nc.dram_tensor("wvt", [D, D], mm_dt, kind="ExternalInput")
    ucol = nc.dram_tensor("ucol", [D, 1], f32, kind="ExternalInput")
    bvb = nc.dram_tensor("bvb", [D, 512], f32, kind="ExternalInput")
    mka = nc.dram_tensor("mka", [32, D], bf16, kind="ExternalInput")
    mkb = nc.dram_tensor("mkb", [32, 4 * D], bf16, kind="ExternalInput")
    one = nc.dram_tensor("one", [D, 1], mm_dt, kind="ExternalInput")
    y = nc.dram_tensor("y", [TOK, F], f32, kind="ExternalOutput")

    xt_r = xt.ap().rearrange("d (T c) -> T d c", c=P * HEADS)
    # DRAM element address for out tile T, group j, partition p=(t%16)*8+h, e:
    # (T*128 + 16j + p//8)*1024 + (p%8)*128 + e = T*131072 + j*16384 + p*128 + e
    if SCR_OUT:
        # scrambled: tile-row-major dump; host un-permutes
        y_r = y.ap().rearrange("(T p) c -> T p c", p=P)
    else:
        y_r = y.ap().flatten().rearrange(
            "(T j p e) -> T p j e", T=NT, j=8, p=P, e=D
        )

    AF = mybir.ActivationFunctionType

    with tile.TileContext(nc) as tc, ExitStack() as es:
        cpool = es.enter_context(tc.tile_pool(name="consts", bufs=1))
        zmt_s = cpool.tile([D, D], mm_dt, tag="zmt")
        wvt_s = cpool.tile([D, D], mm_dt, tag="wvt")
        ucol_s = cpool.tile([D, 1], f32, tag="ucol")
        bvb_s = cpool.tile([D, 512], f32, tag="bvb")
        mka_s = cpool.tile([32, D], bf16, tag="mka")
        mkb_s = cpool.tile([32, 4 * D], bf16, tag="mkb")
        one_s = cpool.tile([D, 1], mm_dt, tag="one")
        for t_, d_ in (
            (zmt_s, zmt), (wvt_s, wvt), (bvb_s, bvb),
            (ucol_s, ucol), (mka_s, mka), (mkb_s, mkb), (one_s, one),
        ):
            nc.scalar.dma_start(t_[:], d_.ap())
        bvb_v = bvb_s[:].rearrange("p (j e) -> p j e", e=D)

        pxt = es.enter_context(tc.tile_pool(name="pxt", bufs=BUFS.get("pxt", 2)))
        pz = es.enter_context(tc.tile_pool(name="pz", bufs=BUFS.get("pz", 2)))
        pv = es.enter_context(tc.tile_pool(name="pv", bufs=BUFS.get("pv", 2)))
        ppt = es.enter_context(tc.tile_pool(name="ppt", bufs=BUFS.get("ppt", 3)))
        pdr = es.enter_context(tc.tile_pool(name="pdr", bufs=BUFS.get("pdr", 2)))
        po = es.enter_context(tc.tile_pool(name="po", bufs=BUFS.get("po", 3)))
        ps = es.enter_context(tc.tile_pool(
            name="ps", bufs=BUFS.get("ps", 5), space="PSUM"))
        pav = es.enter_context(tc.tile_pool(
            name="pav", bufs=BUFS.get("pav", 3), space="PSUM"))
        # warm the ACT exp table while the first DMAs are in flight
        warm = cpool.tile([1, 2], f32, tag="warm")
        nc.gpsimd.memset(warm[:], 0.0)
        nc.scalar.activation(warm[0:1, 0:1], warm[0:1, 1:2], AF.Exp)

        import contextlib
        rep_cm = tc.For_i(0, reps, 1) if reps > 1 else contextlib.nullcontext()
        with rep_cm:
          for T in range(NT):
              XT = pxt.tile([D, P * HEADS], mm_dt, tag="xt")
              if BUFS.get("no_indma"):
                  nc.gpsimd.memset(XT[:], 0.0)
              else:
                  nc.sync.dma_start(XT[:], xt_r[T])
              if BUFS.get("dma_only"):
                  out = po.tile([P, 8, D], f32, tag="o")
                  nc.vector.tensor_copy(out[:, 0, 0:8], XT[:, 0:8])
                  nc.sync.dma_start(y_r[T], out[:])
                  continue

              # ---- z projection: zT2 = (s*Wk^T Wq) x + s*Wk^T bq ----
              # scoresT[(t,g),(t,h)] = x_g . z_h reproduces k.(q*s+bq*s)
              zT2 = pz.tile([D, P * HEADS], mm_dt, tag="z")
              for half in range(2):
                  csl = slice(512 * half, 512 * half + 512)
                  zps = ps.tile([D, 512], f32, tag="ps")
                  nc.tensor.matmul(zps[:], mm(zmt_s[:]), mm(XT[:, csl]),
                                   start=True, stop=True)
                  nc.scalar.activation(zT2[:, csl], zps[:], AF.Identity,
                                       bias=ucol_s[:, 0:1])

              # ---- v projection -> V [(t,g), j, e] ----
              V = pv.tile([P, 8, D], mm_dt, tag="v")
              for half in range(2):
                  vps = ps.tile([P, 4, D], f32, tag="ps")
                  for jj in range(4):
                      j = 4 * half + jj
                      nc.tensor.matmul(vps[:, jj, :],
                                       mm(XT[:, 128 * j:128 * j + 128]),
                                       mm(wvt_s[:]), start=True, stop=True)
                  nc.vector.tensor_add(V[:, 4 * half:4 * half + 4, :], vps[:],
                                       bvb_v[:])

              # ---- scores (transposed) + mask + exp -> PT [(t,g), j, (t,h)] ----
              PT = ppt.tile([P, 8, P], mm_dt, tag="pt")
              for half in range(2):
                  sps = ps.tile([P, 4, P], f32, tag="ps")
                  nc.tensor.matmul(sps[:], mka_s[:], mkb_s[:],
                                   start=True, stop=False)
                  for jj in range(4):
                      j = 4 * half + jj
                      gsl = slice(128 * j, 128 * j + 128)
                      nc.tensor.matmul(sps[:, jj, :], mm(XT[:, gsl]),
                                       mm(zT2[:, gsl]), start=False, stop=True,
                                       skip_group_check=True)
                  nc.scalar.activation(PT[:, 4 * half:4 * half + 4, :], sps[:],
                                       AF.Exp)

              # ---- denominators + AV ----
              dpool = pav if BUFS.get("dps_in_av") else ps
              dtag = "av" if BUFS.get("dps_in_av") else "ps"
              dps = dpool.tile([P, 8], f32, tag=dtag)
              for j in range(8):
                  nc.tensor.matmul(dps[:, j:j + 1], mm(PT[:, j, :]),
                                   mm(one_s[:]), start=True, stop=True)
              rsb = pdr.tile([P, 8], f32, tag="rs")
              nc.vector.reciprocal(rsb[:], dps[:])
              avp = []
              for half in range(2):
                  avps = pav.tile([P, 4, D], f32, tag="av")
                  avp.append(avps)
                  for jj in range(4):
                      j = 4 * half + jj
                      nc.tensor.matmul(avps[:, jj, :], mm(PT[:, j, :]),
                                       mm(V[:, j, :]), start=True, stop=True)

              out = po.tile([P, 8, D], f32, tag="o")
              for half in range(2):
                  hsl = slice(4 * half, 4 * half + 4)
                  nc.vector.tensor_mul(
                      out[:, hsl, :], avp[half][:],
                      rsb[:, hsl, None].broadcast_to([P, 4, D]))
              if SCR_OUT:
                  nc.sync.dma_start(
                      y_r[T], out[:].rearrange("p j e -> p (j e)"))
              else:
                  nc.sync.dma_start(y_r[T], out[:])

    nc.compile()
    return nc


def _get_nc(mm_dt_name="f32"):
    if mm_dt_name not in _NC_CACHE:
        _NC_CACHE[mm_dt_name] = _build_nc(mm_dt_name)
    return _NC_CACHE[mm_dt_name]


def _prep_in_maps(x, Wq, bq, Wk, bk, Wv, bv, mm_dt_name="f32"):
    import ml_dtypes
    if mm_dt_name == "bf16":
        mm_np = ml_dtypes.bfloat16
    else:
        mm_np = np.float32
    s = np.float32(1.0 / np.sqrt(D))
    Wq = np.asarray(Wq, np.float64)
    Wk = np.asarray(Wk, np.float64)
    zmt = np.ascontiguousarray(s * (Wq.T @ Wk)).astype(mm_np)
    ucol = (s * (Wk.T @ np.asarray(bq, np.float64))).reshape(D, 1).astype(
        np.float32)
    wvt = np.ascontiguousarray(np.asarray(Wv).T).astype(mm_np)
    bvb = np.tile(np.asarray(bv).reshape(1, D).astype(np.float32), (D, 4))
    a = np.float32(np.sqrt(-NEG))
    mka = np.zeros((32, D), np.float32)
    mkb = np.zeros((32, D), np.float32)
    mka[0, :] = a
    mkb[0, :] = -a
    for j in range(16):
        mka[1 + j, 8 * j:8 * j + 8] = a
        mkb[1 + j, 8 * j:8 * j + 8] = a
    mka = mka.astype(ml_dtypes.bfloat16)
    mkb = np.tile(mkb, (1, 4)).astype(ml_dtypes.bfloat16)
    one = np.ones((D, 1), mm_np)
    xs = np.asarray(x, np.float32).reshape(B * N, F)
    shared = dict(zmt=zmt, wvt=wvt, ucol=ucol, bvb=bvb, mka=mka,
                  mkb=mkb, one=one)
    in_maps = []
    for c in range(NCORES):
        xc = xs[c * TOK:(c + 1) * TOK]
        # xt[d, t*8+h] = x[t, h*128+d]
        xtc = np.ascontiguousarray(
            xc.reshape(TOK, HEADS, D).transpose(2, 0, 1).reshape(
                D, TOK * HEADS)).astype(mm_np)
        in_maps.append(dict(xt=xtc, **shared))
    return in_maps


def run(x, Wq, bq, Wk, bk, Wv, bv, mm_dt_name="f32", run_bufs=None,
        **run_kw):
    from concourse.bass_utils import run_bass_kernel_spmd

    nc = _build_nc(mm_dt_name, BUFS=run_bufs) if run_bufs else _get_nc(
        mm_dt_name)
    in_maps = _prep_in_maps(x, Wq, bq, Wk, bk, Wv, bv, mm_dt_name)
    res = run_bass_kernel_spmd(nc, in_maps, core_ids=list(range(NCORES)),
                               **run_kw)
    scr = bool((run_bufs or {}).get("scr_out", 1))
    yl = []
    for c in range(NCORES):
        a = np.asarray(res.results[c]["y"], np.float32)
        if scr:
            a = a.reshape(NT, 16, 8, 8, D).transpose(0, 3, 1, 2, 4).reshape(
                TOK, F)
        yl.append(a)
    y = np.concatenate(yl, axis=0).reshape(B, N, F)
    return y, res


def kernel(x, Wq, bq, Wk, bk, Wv, bv):
    y, _ = run(x, Wq, bq, Wk, bk, Wv, bv, mm_dt_name="bf16")
    return y



# revision 2
# speedup vs baseline: 1.0149x; 1.0149x over previous
"""Trainium2 Bass kernel v3 for nn_BlockSelfAttention (attention over 8 heads per token).
Baseline structure + fp8-DoubleRow mask matmul + bf16 output DMA.

Math per token t (32768 tokens total, 1024 features = 8 heads x 128 dims):
  xh = x[t].reshape(8, 128)                     # (h, d)
  q = xh @ Wq.T + bq ; k = xh @ Wk.T + bk ; v = xh @ Wv.T + bv
  scores = (q @ k.T) / sqrt(128)                # (8, 8) attention over heads
  out[t] = softmax(scores, -1) @ v              # -> reshape back to 1024

Identities used:
  * bk drops out (adds a per-row constant to scores -> softmax invariant).
  * 1/sqrt(d) and bq are folded into Wq/bq on the host.
  * bv is added to v rows; since softmax rows sum to 1 the output gets +bv.

Layout strategy (per core: 4096 tokens = 32 tiles of 128 tokens):
  * the host pre-transposes/interleaves x into xt[d, t*8+h] so each SBUF
    tile XT [d=128, 1024] holds 128 tokens with every 128-column block
    covering 16 whole tokens (all 8 heads).
  * q/k projections: matmul(lhsT=WqT [d,e], rhs=XT) -> qT2/kT2 [e, (t*8+h)].
  * v projection: matmul(lhsT=XT block [(d),(t,g)], rhs=WvT) ->
    V [(t,g), e] (natural orientation) per 16-token group.
  * scores (transposed): matmul(lhsT=kT2 block, rhs=qT2 block) ->
    S [(t,g), (t,h)] 128x128 per 16-token group; a rank-17 mask matmul
    pre-accumulates -30000 on off-token-diagonal entries (so exp -> 0).
  * exp on scalar engine (no max subtraction: |scores| < ~10).
  * denominator: matmul(lhsT=P, rhs=ones) -> [(t,h), 1], batched 8/bank.
  * AV: matmul(lhsT=P, rhs=V) -> out [(t,h), e]; multiply by reciprocal
    denominator (per-partition broadcast) and DMA out; the (t,h) x e tile
    maps to contiguous DRAM rows.
"""

import numpy as np

HEADS = 8
D = 128
B, N, F = 8, 4096, 1024
NCORES = 8
TOK = (B * N) // NCORES          # tokens per core
P = 128                          # tokens per tile
NT = TOK // P                    # tiles per core
NEG = -30000.0

_NC_CACHE = {}


def _build_nc(mm_dt_name="f32", BUFS=None, reps=1):
    import concourse.mybir as mybir
    import concourse.tile as tile
    from concourse import bacc
    from contextlib import ExitStack

    f32 = mybir.dt.float32
    bf16 = mybir.dt.bfloat16
    if mm_dt_name in ("f32", "f32r"):
        mm_dt = f32
    elif mm_dt_name == "bf16":
        mm_dt = bf16
    else:
        raise ValueError(mm_dt_name)

    def mm(ap):
        # reinterpret an fp32 AP as fp32r at matmul call sites
        if mm_dt_name == "f32r" and ap.dtype == f32:
            return ap.bitcast(mybir.dt.float32r)
        return ap

    BUFS = BUFS or {}
    SCR_OUT = bool(BUFS.get("scr_out", 1))
    nc = bacc.Bacc("TRN2", target_bir_lowering=False, debug=False)

    xt = nc.dram_tensor("xt", [D, TOK * HEADS], mm_dt, kind="ExternalInput")
    zmt = nc.dram_tensor("zmt", [D, D], mm_dt, kind="ExternalInput")
    wvt = nc.dram_tensor("wvt", [D, D], mm_dt, kind="ExternalInput")
    ucol = nc.dram_tensor("ucol", [D, 1], f32, kind="ExternalInput")
    bvb = nc.dram_tensor("bvb", [D, 512], f32, kind="ExternalInput")
    fp8 = mybir.dt.float8e4
    mka = nc.dram_tensor("mka", [16, 2 * D], fp8, kind="ExternalInput")
    mkb = nc.dram_tensor("mkb", [16, 2 * 4 * D], fp8, kind="ExternalInput")
    one = nc.dram_tensor("one", [D, 1], mm_dt, kind="ExternalInput")
    y = nc.dram_tensor("y", [TOK, F], bf16, kind="ExternalOutput")

    xt_r = xt.ap().rearrange("d (T c) -> T d c", c=P * HEADS)
    # DRAM element address for out tile T, group j, partition p=(t%16)*8+h, e:
    # (T*128 + 16j + p//8)*1024 + (p%8)*128 + e = T*131072 + j*16384 + p*128 + e
    if SCR_OUT:
        # scrambled: tile-row-major dump; host un-permutes
        y_r = y.ap().rearrange("(T p) c -> T p c", p=P)
    else:
        y_r = y.ap().flatten().rearrange(
            "(T j p e) -> T p j e", T=NT, j=8, p=P, e=D
        )

    AF = mybir.ActivationFunctionType

    with tile.TileContext(nc) as tc, ExitStack() as es:
        cpool = es.enter_context(tc.tile_pool(name="consts", bufs=1))
        zmt_s = cpool.tile([D, D], mm_dt, tag="zmt")
        wvt_s = cpool.tile([D, D], mm_dt, tag="wvt")
        ucol_s = cpool.tile([D, 1], f32, tag="ucol")
        bvb_s = cpool.tile([D, 512], f32, tag="bvb")
        mka_s = cpool.tile([16, 2, D], fp8, tag="mka")
        mkb_s = cpool.tile([16, 2, 4 * D], fp8, tag="mkb")
        one_s = cpool.tile([D, 1], mm_dt, tag="one")
        for t_, d_ in (
            (zmt_s, zmt), (wvt_s, wvt), (bvb_s, bvb),
            (ucol_s, ucol), (one_s, one),
        ):
            nc.scalar.dma_start(t_[:], d_.ap())
        nc.scalar.dma_start(
            mka_s[:].rearrange("k two d -> k (two d)"), mka.ap())
        nc.scalar.dma_start(
            mkb_s[:].rearrange("k two d -> k (two d)"), mkb.ap())
        bvb_v = bvb_s[:].rearrange("p (j e) -> p j e", e=D)

        pxt = es.enter_context(tc.tile_pool(name="pxt", bufs=BUFS.get("pxt", 2)))
        pz = es.enter_context(tc.tile_pool(name="pz", bufs=BUFS.get("pz", 2)))
        pv = es.enter_context(tc.tile_pool(name="pv", bufs=BUFS.get("pv", 2)))
        ppt = es.enter_context(tc.tile_pool(name="ppt", bufs=BUFS.get("ppt", 3)))
        pdr = es.enter_context(tc.tile_pool(name="pdr", bufs=BUFS.get("pdr", 2)))
        po = es.enter_context(tc.tile_pool(name="po", bufs=BUFS.get("po", 3)))
        ps = es.enter_context(tc.tile_pool(
            name="ps", bufs=BUFS.get("ps", 5), space="PSUM"))
        pav = es.enter_context(tc.tile_pool(
            name="pav", bufs=BUFS.get("pav", 3), space="PSUM"))
        # warm the ACT exp table while the first DMAs are in flight
        warm = cpool.tile([1, 2], f32, tag="warm")
        nc.gpsimd.memset(warm[:], 0.0)
        nc.scalar.activation(warm[0:1, 0:1], warm[0:1, 1:2], AF.Exp)

        import contextlib
        rep_cm = tc.For_i(0, reps, 1) if reps > 1 else contextlib.nullcontext()
        with rep_cm:
          for T in range(NT):
              XT = pxt.tile([D, P * HEADS], mm_dt, tag="xt")
              if BUFS.get("no_indma"):
                  nc.gpsimd.memset(XT[:], 0.0)
              else:
                  nc.sync.dma_start(XT[:], xt_r[T])
              if BUFS.get("dma_only"):
                  out = po.tile([P, 8, D], bf16, tag="o")
                  nc.vector.tensor_copy(out[:, 0, 0:8], XT[:, 0:8])
                  nc.sync.dma_start(y_r[T], out[:])
                  continue

              # ---- z projection: zT2 = (s*Wk^T Wq) x + s*Wk^T bq ----
              # scoresT[(t,g),(t,h)] = x_g . z_h reproduces k.(q*s+bq*s)
              zT2 = pz.tile([D, P * HEADS], mm_dt, tag="z")
              for half in range(2):
                  csl = slice(512 * half, 512 * half + 512)
                  zps = ps.tile([D, 512], f32, tag="ps")
                  nc.tensor.matmul(zps[:], mm(zmt_s[:]), mm(XT[:, csl]),
                                   start=True, stop=True)
                  nc.scalar.activation(zT2[:, csl], zps[:], AF.Identity,
                                       bias=ucol_s[:, 0:1])

              # ---- v projection -> V [(t,g), j, e] ----
              V = pv.tile([P, 8, D], mm_dt, tag="v")
              for half in range(2):
                  vps = ps.tile([P, 4, D], f32, tag="ps")
                  for jj in range(4):
                      j = 4 * half + jj
                      nc.tensor.matmul(vps[:, jj, :],
                                       mm(XT[:, 128 * j:128 * j + 128]),
                                       mm(wvt_s[:]), start=True, stop=True)
                  nc.vector.tensor_add(V[:, 4 * half:4 * half + 4, :], vps[:],
                                       bvb_v[:])

              # ---- scores (transposed) + mask + exp -> PT [(t,g), j, (t,h)] ----
              PT = ppt.tile([P, 8, P], mm_dt, tag="pt")
              for half in range(2):
                  sps = ps.tile([P, 4, P], f32, tag="ps")
                  nc.tensor.matmul(sps[:], mka_s[:], mkb_s[:],
                                   start=True, stop=False,
                                   perf_mode=mybir.MatmulPerfMode.DoubleRow)
                  for jj in range(4):
                      j = 4 * half + jj
                      gsl = slice(128 * j, 128 * j + 128)
                      nc.tensor.matmul(sps[:, jj, :], mm(XT[:, gsl]),
                                       mm(zT2[:, gsl]), start=False, stop=True,
                                       skip_group_check=True)
                  nc.scalar.activation(PT[:, 4 * half:4 * half + 4, :], sps[:],
                                       AF.Exp)

              # ---- denominators + AV ----
              dpool = pav if BUFS.get("dps_in_av") else ps
              dtag = "av" if BUFS.get("dps_in_av") else "ps"
              dps = dpool.tile([P, 8], f32, tag=dtag)
              for j in range(8):
                  nc.tensor.matmul(dps[:, j:j + 1], mm(PT[:, j, :]),
                                   mm(one_s[:]), start=True, stop=True)
              rsb = pdr.tile([P, 8], f32, tag="rs")
              nc.vector.reciprocal(rsb[:], dps[:])
              avp = []
              for half in range(2):
                  avps = pav.tile([P, 4, D], f32, tag="av")
                  avp.append(avps)
                  for jj in range(4):
                      j = 4 * half + jj
                      nc.tensor.matmul(avps[:, jj, :], mm(PT[:, j, :]),
                                       mm(V[:, j, :]), start=True, stop=True)

              out = po.tile([P, 8, D], bf16, tag="o")
              for half in range(2):
                  hsl = slice(4 * half, 4 * half + 4)
                  nc.vector.tensor_mul(
                      out[:, hsl, :], avp[half][:],
                      rsb[:, hsl, None].broadcast_to([P, 4, D]))
              if SCR_OUT:
                  nc.sync.dma_start(
                      y_r[T], out[:].rearrange("p j e -> p (j e)"))
              else:
                  nc.sync.dma_start(y_r[T], out[:])

    nc.compile()
    return nc


def _get_nc(mm_dt_name="f32"):
    if mm_dt_name not in _NC_CACHE:
        _NC_CACHE[mm_dt_name] = _build_nc(mm_dt_name)
    return _NC_CACHE[mm_dt_name]


def _prep_in_maps(x, Wq, bq, Wk, bk, Wv, bv, mm_dt_name="f32"):
    import ml_dtypes
    if mm_dt_name == "bf16":
        mm_np = ml_dtypes.bfloat16
    else:
        mm_np = np.float32
    s = np.float32(1.0 / np.sqrt(D))
    Wq = np.asarray(Wq, np.float64)
    Wk = np.asarray(Wk, np.float64)
    zmt = np.ascontiguousarray(s * (Wq.T @ Wk)).astype(mm_np)
    ucol = (s * (Wk.T @ np.asarray(bq, np.float64))).reshape(D, 1).astype(
        np.float32)
    wvt = np.ascontiguousarray(np.asarray(Wv).T).astype(mm_np)
    bvb = np.tile(np.asarray(bv).reshape(1, D).astype(np.float32), (D, 4))
    a = np.float32(176.0)   # a*a = 30976; exact in fp8 e4m3
    mka = np.zeros((32, D), np.float32)
    mkb = np.zeros((32, D), np.float32)
    mka[0, :] = a
    mkb[0, :] = -a
    for j in range(16):
        mka[1 + j, 8 * j:8 * j + 8] = a
        mkb[1 + j, 8 * j:8 * j + 8] = a
    mkb = np.tile(mkb, (1, 4))
    # DoubleRow layout [K=16, 2, cols]: result = sum_i A_i^T B_i
    mka = mka.reshape(2, 16, D).transpose(1, 0, 2).reshape(16, 2 * D)
    mkb = mkb.reshape(2, 16, 4 * D).transpose(1, 0, 2).reshape(16, 8 * D)
    mka = mka.astype(ml_dtypes.float8_e4m3)
    mkb = mkb.astype(ml_dtypes.float8_e4m3)
    one = np.ones((D, 1), mm_np)
    xs = np.asarray(x, np.float32).reshape(B * N, F)
    shared = dict(zmt=zmt, wvt=wvt, ucol=ucol, bvb=bvb, mka=mka,
                  mkb=mkb, one=one)
    in_maps = []
    for c in range(NCORES):
        xc = xs[c * TOK:(c + 1) * TOK]
        # xt[d, t*8+h] = x[t, h*128+d]
        xtc = np.ascontiguousarray(
            xc.reshape(TOK, HEADS, D).transpose(2, 0, 1).reshape(
                D, TOK * HEADS)).astype(mm_np)
        in_maps.append(dict(xt=xtc, **shared))
    return in_maps


def run(x, Wq, bq, Wk, bk, Wv, bv, mm_dt_name="f32", run_bufs=None,
        **run_kw):
    from concourse.bass_utils import run_bass_kernel_spmd

    nc = _build_nc(mm_dt_name, BUFS=run_bufs) if run_bufs else _get_nc(
        mm_dt_name)
    in_maps = _prep_in_maps(x, Wq, bq, Wk, bk, Wv, bv, mm_dt_name)
    res = run_bass_kernel_spmd(nc, in_maps, core_ids=list(range(NCORES)),
                               **run_kw)
    scr = bool((run_bufs or {}).get("scr_out", 1))
    yl = []
    for c in range(NCORES):
        a = np.asarray(res.results[c]["y"], np.float32)
        if scr:
            a = a.reshape(NT, 16, 8, 8, D).transpose(0, 3, 1, 2, 4).reshape(
                TOK, F)
        yl.append(a)
    y = np.concatenate(yl, axis=0).reshape(B, N, F)
    return y, res


def kernel(x, Wq, bq, Wk, bk, Wv, bv):
    y, _ = run(x, Wq, bq, Wk, bk, Wv, bv, mm_dt_name="bf16")
    return y



# revision 3
# speedup vs baseline: 1.0306x; 1.0155x over previous
"""Trainium2 Bass kernel v3 for nn_BlockSelfAttention (attention over 8 heads per token).
Baseline structure + fp8-DoubleRow mask matmul + bf16 output DMA.

Math per token t (32768 tokens total, 1024 features = 8 heads x 128 dims):
  xh = x[t].reshape(8, 128)                     # (h, d)
  q = xh @ Wq.T + bq ; k = xh @ Wk.T + bk ; v = xh @ Wv.T + bv
  scores = (q @ k.T) / sqrt(128)                # (8, 8) attention over heads
  out[t] = softmax(scores, -1) @ v              # -> reshape back to 1024

Identities used:
  * bk drops out (adds a per-row constant to scores -> softmax invariant).
  * 1/sqrt(d) and bq are folded into Wq/bq on the host.
  * bv is added to v rows; since softmax rows sum to 1 the output gets +bv.

Layout strategy (per core: 4096 tokens = 32 tiles of 128 tokens):
  * the host pre-transposes/interleaves x into xt[d, t*8+h] so each SBUF
    tile XT [d=128, 1024] holds 128 tokens with every 128-column block
    covering 16 whole tokens (all 8 heads).
  * q/k projections: matmul(lhsT=WqT [d,e], rhs=XT) -> qT2/kT2 [e, (t*8+h)].
  * v projection: matmul(lhsT=XT block [(d),(t,g)], rhs=WvT) ->
    V [(t,g), e] (natural orientation) per 16-token group.
  * scores (transposed): matmul(lhsT=kT2 block, rhs=qT2 block) ->
    S [(t,g), (t,h)] 128x128 per 16-token group; a rank-17 mask matmul
    pre-accumulates -30000 on off-token-diagonal entries (so exp -> 0).
  * exp on scalar engine (no max subtraction: |scores| < ~10).
  * denominator: matmul(lhsT=P, rhs=ones) -> [(t,h), 1], batched 8/bank.
  * AV: matmul(lhsT=P, rhs=V) -> out [(t,h), e]; multiply by reciprocal
    denominator (per-partition broadcast) and DMA out; the (t,h) x e tile
    maps to contiguous DRAM rows.
"""

import numpy as np

HEADS = 8
D = 128
B, N, F = 8, 4096, 1024
NCORES = 8
TOK = (B * N) // NCORES          # tokens per core
P = 128                          # tokens per tile
NT = TOK // P                    # tiles per core
NEG = -30000.0

_NC_CACHE = {}


def _build_nc(mm_dt_name="f32", BUFS=None, reps=1):
    import concourse.mybir as mybir
    import concourse.tile as tile
    from concourse import bacc
    from contextlib import ExitStack

    f32 = mybir.dt.float32
    bf16 = mybir.dt.bfloat16
    if mm_dt_name in ("f32", "f32r"):
        mm_dt = f32
    elif mm_dt_name == "bf16":
        mm_dt = bf16
    else:
        raise ValueError(mm_dt_name)

    def mm(ap):
        # reinterpret an fp32 AP as fp32r at matmul call sites
        if mm_dt_name == "f32r" and ap.dtype == f32:
            return ap.bitcast(mybir.dt.float32r)
        return ap

    BUFS = BUFS or {}
    SCR_OUT = bool(BUFS.get("scr_out", 1))
    nc = bacc.Bacc("TRN2", target_bir_lowering=False, debug=False)

    xt = nc.dram_tensor("xt", [D, TOK * HEADS], mm_dt, kind="ExternalInput")
    zmt = nc.dram_tensor("zmt", [D, D], mm_dt, kind="ExternalInput")
    wvt = nc.dram_tensor("wvt", [D, D], mm_dt, kind="ExternalInput")
    ucol = nc.dram_tensor("ucol", [D, 1], f32, kind="ExternalInput")
    bvb = nc.dram_tensor("bvb", [D, 512], f32, kind="ExternalInput")
    fp8 = mybir.dt.float8e4
    mka = nc.dram_tensor("mka", [16, 2 * D], fp8, kind="ExternalInput")
    mkb = nc.dram_tensor("mkb", [16, 2 * 4 * D], fp8, kind="ExternalInput")
    one = nc.dram_tensor("one", [D, 1], mm_dt, kind="ExternalInput")
    y = nc.dram_tensor("y", [TOK, F], bf16, kind="ExternalOutput")

    xt_r = xt.ap().rearrange("d (T c) -> T d c", c=P * HEADS)
    # DRAM element address for out tile T, group j, partition p=(t%16)*8+h, e:
    # (T*128 + 16j + p//8)*1024 + (p%8)*128 + e = T*131072 + j*16384 + p*128 + e
    if SCR_OUT:
        # scrambled: tile-row-major dump; host un-permutes
        y_r = y.ap().rearrange("(T p) c -> T p c", p=P)
    else:
        y_r = y.ap().flatten().rearrange(
            "(T j p e) -> T p j e", T=NT, j=8, p=P, e=D
        )

    AF = mybir.ActivationFunctionType

    with tile.TileContext(nc) as tc, ExitStack() as es:
        cpool = es.enter_context(tc.tile_pool(name="consts", bufs=1))
        zmt_s = cpool.tile([D, D], mm_dt, tag="zmt")
        wvt_s = cpool.tile([D, D], mm_dt, tag="wvt")
        ucol_s = cpool.tile([D, 1], f32, tag="ucol")
        bvb_s = cpool.tile([D, 512], f32, tag="bvb")
        mka_s = cpool.tile([16, 2, D], fp8, tag="mka")
        mkb_s = cpool.tile([16, 2, 4 * D], fp8, tag="mkb")
        one_s = cpool.tile([D, 1], mm_dt, tag="one")
        for t_, d_ in (
            (zmt_s, zmt), (wvt_s, wvt), (bvb_s, bvb),
            (ucol_s, ucol), (one_s, one),
        ):
            nc.scalar.dma_start(t_[:], d_.ap())
        nc.scalar.dma_start(
            mka_s[:].rearrange("k two d -> k (two d)"), mka.ap())
        nc.scalar.dma_start(
            mkb_s[:].rearrange("k two d -> k (two d)"), mkb.ap())
        bvb_v = bvb_s[:].rearrange("p (j e) -> p j e", e=D)

        pxt = es.enter_context(tc.tile_pool(name="pxt", bufs=BUFS.get("pxt", 3)))
        pz = es.enter_context(tc.tile_pool(name="pz", bufs=BUFS.get("pz", 2)))
        pv = es.enter_context(tc.tile_pool(name="pv", bufs=BUFS.get("pv", 4)))
        ppt = es.enter_context(tc.tile_pool(name="ppt", bufs=BUFS.get("ppt", 4)))
        pdr = es.enter_context(tc.tile_pool(name="pdr", bufs=BUFS.get("pdr", 3)))
        po = es.enter_context(tc.tile_pool(name="po", bufs=BUFS.get("po", 4)))
        ps = es.enter_context(tc.tile_pool(
            name="ps", bufs=BUFS.get("ps", 6), space="PSUM"))
        pav = es.enter_context(tc.tile_pool(
            name="pav", bufs=BUFS.get("pav", 2), space="PSUM"))
        # warm the ACT exp table while the first DMAs are in flight
        warm = cpool.tile([1, 2], f32, tag="warm")
        nc.gpsimd.memset(warm[:], 0.0)
        nc.scalar.activation(warm[0:1, 0:1], warm[0:1, 1:2], AF.Exp)
        WARM_MM = int(BUFS.get("warm_mm", 12))
        if WARM_MM:
            # spin the PE p-state up while the first input DMA is in flight
            wsb = cpool.tile([D, 512], mm_dt, tag="wsb")
            nc.gpsimd.memset(wsb[:], 0.0)
            for w in range(WARM_MM):
                wps = ps.tile([D, 512], f32, tag="ps", name=f"wps{w}")
                nc.tensor.matmul(wps[:], wsb[:, 0:D], wsb[:],
                                 start=True, stop=True)

        pts, vss = {}, {}

        def front(T):
            """DMA in + z/V projections + scores + exp for tile T."""
            XT = pxt.tile([D, P * HEADS], mm_dt, tag="xt", name=f"xt{T}")
            nc.sync.dma_start(XT[:], xt_r[T])

            # ---- z projection: zT2 = (s*Wk^T Wq) x + s*Wk^T bq ----
            # scoresT[(t,g),(t,h)] = x_g . z_h reproduces k.(q*s+bq*s)
            zT2 = pz.tile([D, P * HEADS], mm_dt, tag="z", name=f"z{T}")
            for half in range(2):
                csl = slice(512 * half, 512 * half + 512)
                zps = ps.tile([D, 512], f32, tag="ps", name=f"zps{T}_{half}")
                nc.tensor.matmul(zps[:], mm(zmt_s[:]), mm(XT[:, csl]),
                                 start=True, stop=True)
                nc.scalar.activation(zT2[:, csl], zps[:], AF.Identity,
                                     bias=ucol_s[:, 0:1])

            # ---- v projection -> V [(t,g), j, e] ----
            V = pv.tile([P, 8, D], mm_dt, tag="v", name=f"v{T}")
            vss[T] = V
            for half in range(2):
                vps = ps.tile([P, 4, D], f32, tag="ps", name=f"vps{T}_{half}")
                for jj in range(4):
                    j = 4 * half + jj
                    nc.tensor.matmul(vps[:, jj, :],
                                     mm(XT[:, 128 * j:128 * j + 128]),
                                     mm(wvt_s[:]), start=True, stop=True)
                nc.vector.tensor_add(V[:, 4 * half:4 * half + 4, :], vps[:],
                                     bvb_v[:])

            # ---- scores (transposed) + mask + exp -> PT [(t,g), j, (t,h)] ----
            PT = ppt.tile([P, 8, P], mm_dt, tag="pt", name=f"pt{T}")
            pts[T] = PT
            for half in range(2):
                sps = ps.tile([P, 4, P], f32, tag="ps", name=f"sps{T}_{half}")
                nc.tensor.matmul(sps[:], mka_s[:], mkb_s[:],
                                 start=True, stop=False,
                                 perf_mode=mybir.MatmulPerfMode.DoubleRow)
                for jj in range(4):
                    j = 4 * half + jj
                    gsl = slice(128 * j, 128 * j + 128)
                    nc.tensor.matmul(sps[:, jj, :], mm(XT[:, gsl]),
                                     mm(zT2[:, gsl]), start=False, stop=True,
                                     skip_group_check=True)
                nc.scalar.activation(PT[:, 4 * half:4 * half + 4, :], sps[:],
                                     AF.Exp)

        def back(T):
            """denominators + AV + out for tile T (runs one iter later so
            the exp -> denominator -> AV chain never stalls the PE)."""
            if T < 0 or T >= NT:
                return
            PT = pts.pop(T)
            V = vss.pop(T)
            dps = ps.tile([P, 8], f32, tag="ps", name=f"dps{T}")
            for j in range(8):
                nc.tensor.matmul(dps[:, j:j + 1], mm(PT[:, j, :]),
                                 mm(one_s[:]), start=True, stop=True)
            rsb = pdr.tile([P, 8], f32, tag="rs", name=f"rs{T}")
            nc.vector.reciprocal(rsb[:], dps[:])
            out = po.tile([P, 8, D], bf16, tag="o", name=f"o{T}")
            for half in range(2):
                avps = pav.tile([P, 4, D], f32, tag="av", name=f"av{T}_{half}")
                for jj in range(4):
                    j = 4 * half + jj
                    nc.tensor.matmul(avps[:, jj, :], mm(PT[:, j, :]),
                                     mm(V[:, j, :]), start=True, stop=True)
                hsl = slice(4 * half, 4 * half + 4)
                nc.vector.tensor_mul(
                    out[:, hsl, :], avps[:],
                    rsb[:, hsl, None].broadcast_to([P, 4, D]))
            nc.sync.dma_start(y_r[T], out[:].rearrange("p j e -> p (j e)"))

        LAG = int(BUFS.get("lag", 1))
        BACKFIRST = bool(BUFS.get("backfirst", 0))
        for T in range(NT):
            if BACKFIRST:
                back(T - LAG)
                front(T)
            else:
                front(T)
                back(T - LAG)
        for T in range(NT - LAG, NT):
            back(T)

    nc.compile()
    return nc


def _get_nc(mm_dt_name="f32"):
    if mm_dt_name not in _NC_CACHE:
        _NC_CACHE[mm_dt_name] = _build_nc(mm_dt_name)
    return _NC_CACHE[mm_dt_name]


def _prep_in_maps(x, Wq, bq, Wk, bk, Wv, bv, mm_dt_name="f32"):
    import ml_dtypes
    if mm_dt_name == "bf16":
        mm_np = ml_dtypes.bfloat16
    else:
        mm_np = np.float32
    s = np.float32(1.0 / np.sqrt(D))
    Wq = np.asarray(Wq, np.float64)
    Wk = np.asarray(Wk, np.float64)
    zmt = np.ascontiguousarray(s * (Wq.T @ Wk)).astype(mm_np)
    ucol = (s * (Wk.T @ np.asarray(bq, np.float64))).reshape(D, 1).astype(
        np.float32)
    wvt = np.ascontiguousarray(np.asarray(Wv).T).astype(mm_np)
    bvb = np.tile(np.asarray(bv).reshape(1, D).astype(np.float32), (D, 4))
    a = np.float32(176.0)   # a*a = 30976; exact in fp8 e4m3
    mka = np.zeros((32, D), np.float32)
    mkb = np.zeros((32, D), np.float32)
    mka[0, :] = a
    mkb[0, :] = -a
    for j in range(16):
        mka[1 + j, 8 * j:8 * j + 8] = a
        mkb[1 + j, 8 * j:8 * j + 8] = a
    mkb = np.tile(mkb, (1, 4))
    # DoubleRow layout [K=16, 2, cols]: result = sum_i A_i^T B_i
    mka = mka.reshape(2, 16, D).transpose(1, 0, 2).reshape(16, 2 * D)
    mkb = mkb.reshape(2, 16, 4 * D).transpose(1, 0, 2).reshape(16, 8 * D)
    mka = mka.astype(ml_dtypes.float8_e4m3)
    mkb = mkb.astype(ml_dtypes.float8_e4m3)
    one = np.ones((D, 1), mm_np)
    xs = np.asarray(x, np.float32).reshape(B * N, F)
    shared = dict(zmt=zmt, wvt=wvt, ucol=ucol, bvb=bvb, mka=mka,
                  mkb=mkb, one=one)
    in_maps = []
    for c in range(NCORES):
        xc = xs[c * TOK:(c + 1) * TOK]
        # xt[d, t*8+h] = x[t, h*128+d]
        xtc = np.ascontiguousarray(
            xc.reshape(TOK, HEADS, D).transpose(2, 0, 1).reshape(
                D, TOK * HEADS)).astype(mm_np)
        in_maps.append(dict(xt=xtc, **shared))
    return in_maps


def run(x, Wq, bq, Wk, bk, Wv, bv, mm_dt_name="f32", run_bufs=None,
        **run_kw):
    from concourse.bass_utils import run_bass_kernel_spmd

    nc = _build_nc(mm_dt_name, BUFS=run_bufs) if run_bufs else _get_nc(
        mm_dt_name)
    in_maps = _prep_in_maps(x, Wq, bq, Wk, bk, Wv, bv, mm_dt_name)
    res = run_bass_kernel_spmd(nc, in_maps, core_ids=list(range(NCORES)),
                               **run_kw)
    scr = bool((run_bufs or {}).get("scr_out", 1))
    yl = []
    for c in range(NCORES):
        a = np.asarray(res.results[c]["y"], np.float32)
        if scr:
            a = a.reshape(NT, 16, 8, 8, D).transpose(0, 3, 1, 2, 4).reshape(
                TOK, F)
        yl.append(a)
    y = np.concatenate(yl, axis=0).reshape(B, N, F)
    return y, res


def kernel(x, Wq, bq, Wk, bk, Wv, bv):
    y, _ = run(x, Wq, bq, Wk, bk, Wv, bv, mm_dt_name="bf16")
    return y



# revision 4
# speedup vs baseline: 1.0522x; 1.0209x over previous
"""Trainium2 Bass kernel v3 for nn_BlockSelfAttention (attention over 8 heads per token).
Baseline structure + fp8-DoubleRow mask matmul + bf16 output DMA.

Math per token t (32768 tokens total, 1024 features = 8 heads x 128 dims):
  xh = x[t].reshape(8, 128)                     # (h, d)
  q = xh @ Wq.T + bq ; k = xh @ Wk.T + bk ; v = xh @ Wv.T + bv
  scores = (q @ k.T) / sqrt(128)                # (8, 8) attention over heads
  out[t] = softmax(scores, -1) @ v              # -> reshape back to 1024

Identities used:
  * bk drops out (adds a per-row constant to scores -> softmax invariant).
  * 1/sqrt(d) and bq are folded into Wq/bq on the host.
  * bv is added to v rows; since softmax rows sum to 1 the output gets +bv.

Layout strategy (per core: 4096 tokens = 32 tiles of 128 tokens):
  * the host pre-transposes/interleaves x into xt[d, t*8+h] so each SBUF
    tile XT [d=128, 1024] holds 128 tokens with every 128-column block
    covering 16 whole tokens (all 8 heads).
  * q/k projections: matmul(lhsT=WqT [d,e], rhs=XT) -> qT2/kT2 [e, (t*8+h)].
  * v projection: matmul(lhsT=XT block [(d),(t,g)], rhs=WvT) ->
    V [(t,g), e] (natural orientation) per 16-token group.
  * scores (transposed): matmul(lhsT=kT2 block, rhs=qT2 block) ->
    S [(t,g), (t,h)] 128x128 per 16-token group; a rank-17 mask matmul
    pre-accumulates -30000 on off-token-diagonal entries (so exp -> 0).
  * exp on scalar engine (no max subtraction: |scores| < ~10).
  * denominator: matmul(lhsT=P, rhs=ones) -> [(t,h), 1], batched 8/bank.
  * AV: matmul(lhsT=P, rhs=V) -> out [(t,h), e]; multiply by reciprocal
    denominator (per-partition broadcast) and DMA out; the (t,h) x e tile
    maps to contiguous DRAM rows.
"""

import numpy as np

HEADS = 8
D = 128
B, N, F = 8, 4096, 1024
NCORES = 8
TOK = (B * N) // NCORES          # tokens per core
P = 128                          # tokens per tile
NT = TOK // P                    # tiles per core
NEG = -30000.0

_NC_CACHE = {}


def _build_nc(mm_dt_name="f32", BUFS=None, reps=1):
    import concourse.mybir as mybir
    import concourse.tile as tile
    from concourse import bacc
    from contextlib import ExitStack

    f32 = mybir.dt.float32
    bf16 = mybir.dt.bfloat16
    if mm_dt_name in ("f32", "f32r"):
        mm_dt = f32
    elif mm_dt_name == "bf16":
        mm_dt = bf16
    else:
        raise ValueError(mm_dt_name)

    def mm(ap):
        # reinterpret an fp32 AP as fp32r at matmul call sites
        if mm_dt_name == "f32r" and ap.dtype == f32:
            return ap.bitcast(mybir.dt.float32r)
        return ap

    BUFS = BUFS or {}
    SCR_OUT = bool(BUFS.get("scr_out", 1))
    nc = bacc.Bacc("TRN2", target_bir_lowering=False, debug=False)

    xt = nc.dram_tensor("xt", [D, TOK * HEADS], mm_dt, kind="ExternalInput")
    zmt = nc.dram_tensor("zmt", [D, D], mm_dt, kind="ExternalInput")
    wvt = nc.dram_tensor("wvt", [D, D], mm_dt, kind="ExternalInput")
    ucol = nc.dram_tensor("ucol", [D, 1], f32, kind="ExternalInput")
    bvb = nc.dram_tensor("bvb", [D, 512], f32, kind="ExternalInput")
    fp8 = mybir.dt.float8e4
    mka = nc.dram_tensor("mka", [16, 2 * D], fp8, kind="ExternalInput")
    mkb = nc.dram_tensor("mkb", [16, 2 * 4 * D], fp8, kind="ExternalInput")
    one = nc.dram_tensor("one", [D, 1], mm_dt, kind="ExternalInput")
    y = nc.dram_tensor("y", [TOK, F], bf16, kind="ExternalOutput")

    xt_r = xt.ap().rearrange("d (T c) -> T d c", c=P * HEADS)
    # DRAM element address for out tile T, group j, partition p=(t%16)*8+h, e:
    # (T*128 + 16j + p//8)*1024 + (p%8)*128 + e = T*131072 + j*16384 + p*128 + e
    if SCR_OUT:
        # scrambled: tile-row-major dump; host un-permutes
        y_r = y.ap().rearrange("(T p) c -> T p c", p=P)
    else:
        y_r = y.ap().flatten().rearrange(
            "(T j p e) -> T p j e", T=NT, j=8, p=P, e=D
        )

    AF = mybir.ActivationFunctionType

    with tile.TileContext(nc) as tc, ExitStack() as es:
        cpool = es.enter_context(tc.tile_pool(name="consts", bufs=1))
        zmt_s = cpool.tile([D, D], mm_dt, tag="zmt")
        wvt_s = cpool.tile([D, D], mm_dt, tag="wvt")
        ucol_s = cpool.tile([D, 1], f32, tag="ucol")
        bvb_s = cpool.tile([D, 512], f32, tag="bvb")
        mka_s = cpool.tile([16, 2, D], fp8, tag="mka")
        mkb_s = cpool.tile([16, 2, 4 * D], fp8, tag="mkb")
        one_s = cpool.tile([D, 1], mm_dt, tag="one")
        for t_, d_ in (
            (zmt_s, zmt), (wvt_s, wvt), (bvb_s, bvb),
            (ucol_s, ucol), (one_s, one),
        ):
            nc.scalar.dma_start(t_[:], d_.ap())
        nc.scalar.dma_start(
            mka_s[:].rearrange("k two d -> k (two d)"), mka.ap())
        nc.scalar.dma_start(
            mkb_s[:].rearrange("k two d -> k (two d)"), mkb.ap())
        bvb_v = bvb_s[:].rearrange("p (j e) -> p j e", e=D)

        pxt = es.enter_context(tc.tile_pool(name="pxt", bufs=BUFS.get("pxt", 3)))
        pz = es.enter_context(tc.tile_pool(name="pz", bufs=BUFS.get("pz", 2)))
        pv = es.enter_context(tc.tile_pool(name="pv", bufs=BUFS.get("pv", 4)))
        ppt = es.enter_context(tc.tile_pool(name="ppt", bufs=BUFS.get("ppt", 4)))
        pdr = es.enter_context(tc.tile_pool(name="pdr", bufs=BUFS.get("pdr", 3)))
        po = es.enter_context(tc.tile_pool(name="po", bufs=BUFS.get("po", 4)))
        ps = es.enter_context(tc.tile_pool(
            name="ps", bufs=BUFS.get("ps", 6), space="PSUM"))
        pav = es.enter_context(tc.tile_pool(
            name="pav", bufs=BUFS.get("pav", 1), space="PSUM"))
        # warm the ACT exp table while the first DMAs are in flight
        warm = cpool.tile([1, 2], f32, tag="warm")
        nc.gpsimd.memset(warm[:], 0.0)
        nc.scalar.activation(warm[0:1, 0:1], warm[0:1, 1:2], AF.Exp)
        WARM_MM = int(BUFS.get("warm_mm", 16))
        if WARM_MM:
            # spin the PE p-state up while the first input DMA is in flight
            wsb = cpool.tile([D, 512], mm_dt, tag="wsb")
            nc.gpsimd.memset(wsb[:], 0.0)
            for w in range(WARM_MM):
                wps = ps.tile([D, 512], f32, tag="ps", name=f"wps{w}")
                nc.tensor.matmul(wps[:], wsb[:, 0:D], wsb[:],
                                 start=True, stop=True)

        pts, vss = {}, {}

        def front(T):
            """DMA in + z/V projections + scores + exp for tile T."""
            XT = pxt.tile([D, P * HEADS], mm_dt, tag="xt", name=f"xt{T}")
            nc.sync.dma_start(XT[:], xt_r[T])

            # ---- z projection: zT2 = (s*Wk^T Wq) x + s*Wk^T bq ----
            # scoresT[(t,g),(t,h)] = x_g . z_h reproduces k.(q*s+bq*s)
            zT2 = pz.tile([D, P * HEADS], mm_dt, tag="z", name=f"z{T}")
            for half in range(2):
                csl = slice(512 * half, 512 * half + 512)
                zps = ps.tile([D, 512], f32, tag="ps", name=f"zps{T}_{half}")
                nc.tensor.matmul(zps[:], mm(zmt_s[:]), mm(XT[:, csl]),
                                 start=True, stop=True)
                nc.scalar.activation(zT2[:, csl], zps[:], AF.Identity,
                                     bias=ucol_s[:, 0:1])

            # ---- v projection -> V [(t,g), j, e] ----
            V = pv.tile([P, 8, D], mm_dt, tag="v", name=f"v{T}")
            vss[T] = V
            for half in range(2):
                vps = ps.tile([P, 4, D], f32, tag="ps", name=f"vps{T}_{half}")
                for jj in range(4):
                    j = 4 * half + jj
                    nc.tensor.matmul(vps[:, jj, :],
                                     mm(XT[:, 128 * j:128 * j + 128]),
                                     mm(wvt_s[:]), start=True, stop=True)
                nc.vector.tensor_add(V[:, 4 * half:4 * half + 4, :], vps[:],
                                     bvb_v[:])

            # ---- scores (transposed) + mask + exp -> PT [(t,g), j, (t,h)] ----
            PT = ppt.tile([P, 8, P], mm_dt, tag="pt", name=f"pt{T}")
            pts[T] = PT
            for half in range(2):
                sps = ps.tile([P, 4, P], f32, tag="ps", name=f"sps{T}_{half}")
                nc.tensor.matmul(sps[:], mka_s[:], mkb_s[:],
                                 start=True, stop=False,
                                 perf_mode=mybir.MatmulPerfMode.DoubleRow)
                for jj in range(4):
                    j = 4 * half + jj
                    gsl = slice(128 * j, 128 * j + 128)
                    nc.tensor.matmul(sps[:, jj, :], mm(XT[:, gsl]),
                                     mm(zT2[:, gsl]), start=False, stop=True,
                                     skip_group_check=True)
                nc.scalar.activation(PT[:, 4 * half:4 * half + 4, :], sps[:],
                                     AF.Exp)

        def back(T):
            """denominators + AV + out for tile T (runs one iter later so
            the exp -> denominator -> AV chain never stalls the PE)."""
            if T < 0 or T >= NT:
                return
            PT = pts.pop(T)
            V = vss.pop(T)
            dps = ps.tile([P, 8], f32, tag="ps", name=f"dps{T}")
            for j in range(8):
                nc.tensor.matmul(dps[:, j:j + 1], mm(PT[:, j, :]),
                                 mm(one_s[:]), start=True, stop=True)
            rsb = pdr.tile([P, 8], f32, tag="rs", name=f"rs{T}")
            nc.vector.reciprocal(rsb[:], dps[:])
            out = po.tile([P, 8, D], bf16, tag="o", name=f"o{T}")
            avps = pav.tile([P, 8, D], f32, tag="av", name=f"av{T}")
            for j in range(8):
                nc.tensor.matmul(avps[:, j, :], mm(PT[:, j, :]),
                                 mm(V[:, j, :]), start=True, stop=True)
            if T >= NT - 2:
                # drain the tail: DMA each half as soon as it is scaled
                for half in range(2):
                    hsl = slice(4 * half, 4 * half + 4)
                    nc.vector.tensor_mul(
                        out[:, hsl, :], avps[:, hsl, :],
                        rsb[:, hsl, None].broadcast_to([P, 4, D]))
                    nc.sync.dma_start(
                        y_r[T][:, 512 * half:512 * half + 512],
                        out[:, hsl, :].rearrange("p j e -> p (j e)"))
            else:
                nc.vector.tensor_mul(
                    out[:], avps[:],
                    rsb[:, :, None].broadcast_to([P, 8, D]))
                nc.sync.dma_start(
                    y_r[T], out[:].rearrange("p j e -> p (j e)"))

        LAG = int(BUFS.get("lag", 1))
        BACKFIRST = bool(BUFS.get("backfirst", 0))
        for T in range(NT):
            if BACKFIRST:
                back(T - LAG)
                front(T)
            else:
                front(T)
                back(T - LAG)
        for T in range(NT - LAG, NT):
            back(T)

    nc.compile()
    return nc


def _get_nc(mm_dt_name="f32"):
    if mm_dt_name not in _NC_CACHE:
        _NC_CACHE[mm_dt_name] = _build_nc(mm_dt_name)
    return _NC_CACHE[mm_dt_name]


def _prep_in_maps(x, Wq, bq, Wk, bk, Wv, bv, mm_dt_name="f32"):
    import ml_dtypes
    if mm_dt_name == "bf16":
        mm_np = ml_dtypes.bfloat16
    else:
        mm_np = np.float32
    s = np.float32(1.0 / np.sqrt(D))
    Wq = np.asarray(Wq, np.float64)
    Wk = np.asarray(Wk, np.float64)
    zmt = np.ascontiguousarray(s * (Wq.T @ Wk)).astype(mm_np)
    ucol = (s * (Wk.T @ np.asarray(bq, np.float64))).reshape(D, 1).astype(
        np.float32)
    wvt = np.ascontiguousarray(np.asarray(Wv).T).astype(mm_np)
    bvb = np.tile(np.asarray(bv).reshape(1, D).astype(np.float32), (D, 4))
    a = np.float32(176.0)   # a*a = 30976; exact in fp8 e4m3
    mka = np.zeros((32, D), np.float32)
    mkb = np.zeros((32, D), np.float32)
    mka[0, :] = a
    mkb[0, :] = -a
    for j in range(16):
        mka[1 + j, 8 * j:8 * j + 8] = a
        mkb[1 + j, 8 * j:8 * j + 8] = a
    mkb = np.tile(mkb, (1, 4))
    # DoubleRow layout [K=16, 2, cols]: result = sum_i A_i^T B_i
    mka = mka.reshape(2, 16, D).transpose(1, 0, 2).reshape(16, 2 * D)
    mkb = mkb.reshape(2, 16, 4 * D).transpose(1, 0, 2).reshape(16, 8 * D)
    mka = mka.astype(ml_dtypes.float8_e4m3)
    mkb = mkb.astype(ml_dtypes.float8_e4m3)
    one = np.ones((D, 1), mm_np)
    xs = np.asarray(x, np.float32).reshape(B * N, F)
    shared = dict(zmt=zmt, wvt=wvt, ucol=ucol, bvb=bvb, mka=mka,
                  mkb=mkb, one=one)
    in_maps = []
    for c in range(NCORES):
        xc = xs[c * TOK:(c + 1) * TOK]
        # xt[d, t*8+h] = x[t, h*128+d]
        xtc = np.ascontiguousarray(
            xc.reshape(TOK, HEADS, D).transpose(2, 0, 1).reshape(
                D, TOK * HEADS)).astype(mm_np)
        in_maps.append(dict(xt=xtc, **shared))
    return in_maps


def run(x, Wq, bq, Wk, bk, Wv, bv, mm_dt_name="f32", run_bufs=None,
        **run_kw):
    from concourse.bass_utils import run_bass_kernel_spmd

    nc = _build_nc(mm_dt_name, BUFS=run_bufs) if run_bufs else _get_nc(
        mm_dt_name)
    in_maps = _prep_in_maps(x, Wq, bq, Wk, bk, Wv, bv, mm_dt_name)
    res = run_bass_kernel_spmd(nc, in_maps, core_ids=list(range(NCORES)),
                               **run_kw)
    scr = bool((run_bufs or {}).get("scr_out", 1))
    yl = []
    for c in range(NCORES):
        a = np.asarray(res.results[c]["y"], np.float32)
        if scr:
            a = a.reshape(NT, 16, 8, 8, D).transpose(0, 3, 1, 2, 4).reshape(
                TOK, F)
        yl.append(a)
    y = np.concatenate(yl, axis=0).reshape(B, N, F)
    return y, res


def kernel(x, Wq, bq, Wk, bk, Wv, bv):
    y, _ = run(x, Wq, bq, Wk, bk, Wv, bv, mm_dt_name="bf16")
    return y



# revision 5
# speedup vs baseline: 1.0568x; 1.0044x over previous
"""Trainium2 Bass kernel v3 for nn_BlockSelfAttention (attention over 8 heads per token).
Baseline structure + fp8-DoubleRow mask matmul + bf16 output DMA.

Math per token t (32768 tokens total, 1024 features = 8 heads x 128 dims):
  xh = x[t].reshape(8, 128)                     # (h, d)
  q = xh @ Wq.T + bq ; k = xh @ Wk.T + bk ; v = xh @ Wv.T + bv
  scores = (q @ k.T) / sqrt(128)                # (8, 8) attention over heads
  out[t] = softmax(scores, -1) @ v              # -> reshape back to 1024

Identities used:
  * bk drops out (adds a per-row constant to scores -> softmax invariant).
  * 1/sqrt(d) and bq are folded into Wq/bq on the host.
  * bv is added to v rows; since softmax rows sum to 1 the output gets +bv.

Layout strategy (per core: 4096 tokens = 32 tiles of 128 tokens):
  * the host pre-transposes/interleaves x into xt[d, t*8+h] so each SBUF
    tile XT [d=128, 1024] holds 128 tokens with every 128-column block
    covering 16 whole tokens (all 8 heads).
  * q/k projections: matmul(lhsT=WqT [d,e], rhs=XT) -> qT2/kT2 [e, (t*8+h)].
  * v projection: matmul(lhsT=XT block [(d),(t,g)], rhs=WvT) ->
    V [(t,g), e] (natural orientation) per 16-token group.
  * scores (transposed): matmul(lhsT=kT2 block, rhs=qT2 block) ->
    S [(t,g), (t,h)] 128x128 per 16-token group; a rank-17 mask matmul
    pre-accumulates -30000 on off-token-diagonal entries (so exp -> 0).
  * exp on scalar engine (no max subtraction: |scores| < ~10).
  * denominator: matmul(lhsT=P, rhs=ones) -> [(t,h), 1], batched 8/bank.
  * AV: matmul(lhsT=P, rhs=V) -> out [(t,h), e]; multiply by reciprocal
    denominator (per-partition broadcast) and DMA out; the (t,h) x e tile
    maps to contiguous DRAM rows.
"""

import numpy as np

HEADS = 8
D = 128
B, N, F = 8, 4096, 1024
NCORES = 8
TOK = (B * N) // NCORES          # tokens per core
P = 128                          # tokens per tile
NT = TOK // P                    # tiles per core
NEG = -30000.0

_NC_CACHE = {}


def _build_nc(mm_dt_name="f32", BUFS=None, reps=1):
    import concourse.mybir as mybir
    import concourse.tile as tile
    from concourse import bacc
    from contextlib import ExitStack

    f32 = mybir.dt.float32
    bf16 = mybir.dt.bfloat16
    if mm_dt_name in ("f32", "f32r"):
        mm_dt = f32
    elif mm_dt_name == "bf16":
        mm_dt = bf16
    else:
        raise ValueError(mm_dt_name)

    def mm(ap):
        # reinterpret an fp32 AP as fp32r at matmul call sites
        if mm_dt_name == "f32r" and ap.dtype == f32:
            return ap.bitcast(mybir.dt.float32r)
        return ap

    BUFS = BUFS or {}
    SCR_OUT = bool(BUFS.get("scr_out", 1))
    nc = bacc.Bacc("TRN2", target_bir_lowering=False, debug=False)

    xt = nc.dram_tensor("xt", [D, TOK * HEADS], mm_dt, kind="ExternalInput")
    zmt = nc.dram_tensor("zmt", [D, D], mm_dt, kind="ExternalInput")
    wvt = nc.dram_tensor("wvt", [D, D], mm_dt, kind="ExternalInput")
    ucol = nc.dram_tensor("ucol", [D, 1], f32, kind="ExternalInput")
    bvb = nc.dram_tensor("bvb", [D, 512], f32, kind="ExternalInput")
    fp8 = mybir.dt.float8e4
    mka = nc.dram_tensor("mka", [16, 2 * D], fp8, kind="ExternalInput")
    mkb = nc.dram_tensor("mkb", [16, 2 * 4 * D], fp8, kind="ExternalInput")
    one = nc.dram_tensor("one", [D, 1], mm_dt, kind="ExternalInput")
    y = nc.dram_tensor("y", [TOK, F], bf16, kind="ExternalOutput")

    xt_r = xt.ap().rearrange("d (T c) -> T d c", c=P * HEADS)
    # DRAM element address for out tile T, group j, partition p=(t%16)*8+h, e:
    # (T*128 + 16j + p//8)*1024 + (p%8)*128 + e = T*131072 + j*16384 + p*128 + e
    if SCR_OUT:
        # scrambled: tile-row-major dump; host un-permutes
        y_r = y.ap().rearrange("(T p) c -> T p c", p=P)
    else:
        y_r = y.ap().flatten().rearrange(
            "(T j p e) -> T p j e", T=NT, j=8, p=P, e=D
        )

    AF = mybir.ActivationFunctionType

    with tile.TileContext(nc) as tc, ExitStack() as es:
        cpool = es.enter_context(tc.tile_pool(name="consts", bufs=1))
        zmt_s = cpool.tile([D, D], mm_dt, tag="zmt")
        wvt_s = cpool.tile([D, D], mm_dt, tag="wvt")
        ucol_s = cpool.tile([D, 1], f32, tag="ucol")
        bvb_s = cpool.tile([D, 512], f32, tag="bvb")
        mka_s = cpool.tile([16, 2, D], fp8, tag="mka")
        mkb_s = cpool.tile([16, 2, 4 * D], fp8, tag="mkb")
        one_s = cpool.tile([D, 1], mm_dt, tag="one")
        for t_, d_ in (
            (zmt_s, zmt), (wvt_s, wvt), (bvb_s, bvb),
            (ucol_s, ucol), (one_s, one),
        ):
            nc.scalar.dma_start(t_[:], d_.ap())
        nc.scalar.dma_start(
            mka_s[:].rearrange("k two d -> k (two d)"), mka.ap())
        nc.scalar.dma_start(
            mkb_s[:].rearrange("k two d -> k (two d)"), mkb.ap())
        bvb_v = bvb_s[:].rearrange("p (j e) -> p j e", e=D)

        pxt = es.enter_context(tc.tile_pool(name="pxt", bufs=BUFS.get("pxt", 3)))
        pz = es.enter_context(tc.tile_pool(name="pz", bufs=BUFS.get("pz", 2)))
        pv = es.enter_context(tc.tile_pool(name="pv", bufs=BUFS.get("pv", 4)))
        ppt = es.enter_context(tc.tile_pool(name="ppt", bufs=BUFS.get("ppt", 4)))
        pdr = es.enter_context(tc.tile_pool(name="pdr", bufs=BUFS.get("pdr", 3)))
        po = es.enter_context(tc.tile_pool(name="po", bufs=BUFS.get("po", 4)))
        ps = es.enter_context(tc.tile_pool(
            name="ps", bufs=BUFS.get("ps", 6), space="PSUM"))
        pav = es.enter_context(tc.tile_pool(
            name="pav", bufs=BUFS.get("pav", 1), space="PSUM"))
        # warm the ACT exp table while the first DMAs are in flight
        warm = cpool.tile([1, 2], f32, tag="warm")
        nc.gpsimd.memset(warm[:], 0.0)
        nc.scalar.activation(warm[0:1, 0:1], warm[0:1, 1:2], AF.Exp)
        WARM_MM = int(BUFS.get("warm_mm", 14))
        if WARM_MM:
            # spin the PE p-state up while the first input DMA is in flight
            wsb = cpool.tile([D, 512], mm_dt, tag="wsb")
            nc.gpsimd.memset(wsb[:], 0.0)
            for w in range(WARM_MM):
                wps = ps.tile([D, 512], f32, tag="ps", name=f"wps{w}")
                nc.tensor.matmul(wps[:], wsb[:, 0:D], wsb[:],
                                 start=True, stop=True)

        pts, vss = {}, {}

        def front(T):
            """DMA in + z/V projections + scores + exp for tile T."""
            XT = pxt.tile([D, P * HEADS], mm_dt, tag="xt", name=f"xt{T}")
            nc.sync.dma_start(XT[:], xt_r[T])

            # ---- z projection: zT2 = (s*Wk^T Wq) x + s*Wk^T bq ----
            # scoresT[(t,g),(t,h)] = x_g . z_h reproduces k.(q*s+bq*s)
            zT2 = pz.tile([D, P * HEADS], mm_dt, tag="z", name=f"z{T}")
            for half in range(2):
                csl = slice(512 * half, 512 * half + 512)
                zps = ps.tile([D, 512], f32, tag="ps", name=f"zps{T}_{half}")
                nc.tensor.matmul(zps[:], mm(zmt_s[:]), mm(XT[:, csl]),
                                 start=True, stop=True)
                nc.scalar.activation(zT2[:, csl], zps[:], AF.Identity,
                                     bias=ucol_s[:, 0:1])

            # ---- v projection -> V [(t,g), j, e] ----
            V = pv.tile([P, 8, D], mm_dt, tag="v", name=f"v{T}")
            vss[T] = V
            for half in range(2):
                vps = ps.tile([P, 4, D], f32, tag="ps", name=f"vps{T}_{half}")
                for jj in range(4):
                    j = 4 * half + jj
                    nc.tensor.matmul(vps[:, jj, :],
                                     mm(XT[:, 128 * j:128 * j + 128]),
                                     mm(wvt_s[:]), start=True, stop=True)
                nc.vector.tensor_add(V[:, 4 * half:4 * half + 4, :], vps[:],
                                     bvb_v[:])

            # ---- scores (transposed) + mask + exp -> PT [(t,g), j, (t,h)] ----
            PT = ppt.tile([P, 8, P], mm_dt, tag="pt", name=f"pt{T}")
            pts[T] = PT
            for half in range(2):
                sps = ps.tile([P, 4, P], f32, tag="ps", name=f"sps{T}_{half}")
                nc.tensor.matmul(sps[:], mka_s[:], mkb_s[:],
                                 start=True, stop=False,
                                 perf_mode=mybir.MatmulPerfMode.DoubleRow)
                for jj in range(4):
                    j = 4 * half + jj
                    gsl = slice(128 * j, 128 * j + 128)
                    nc.tensor.matmul(sps[:, jj, :], mm(XT[:, gsl]),
                                     mm(zT2[:, gsl]), start=False, stop=True,
                                     skip_group_check=True)
                nc.scalar.activation(PT[:, 4 * half:4 * half + 4, :], sps[:],
                                     AF.Exp)

        def back(T):
            """denominators + AV + out for tile T (runs one iter later so
            the exp -> denominator -> AV chain never stalls the PE)."""
            if T < 0 or T >= NT:
                return
            PT = pts.pop(T)
            V = vss.pop(T)
            dps = ps.tile([P, 8], f32, tag="ps", name=f"dps{T}")
            for j in range(8):
                nc.tensor.matmul(dps[:, j:j + 1], mm(PT[:, j, :]),
                                 mm(one_s[:]), start=True, stop=True)
            rsb = pdr.tile([P, 8], f32, tag="rs", name=f"rs{T}")
            nc.vector.reciprocal(rsb[:], dps[:])
            out = po.tile([P, 8, D], bf16, tag="o", name=f"o{T}")
            avps = pav.tile([P, 8, D], f32, tag="av", name=f"av{T}")
            for j in range(8):
                nc.tensor.matmul(avps[:, j, :], mm(PT[:, j, :]),
                                 mm(V[:, j, :]), start=True, stop=True)
            if T >= NT - int(BUFS.get("tailn", 1)):
                # drain the tail: DMA each half as soon as it is scaled
                for half in range(2):
                    hsl = slice(4 * half, 4 * half + 4)
                    nc.vector.tensor_mul(
                        out[:, hsl, :], avps[:, hsl, :],
                        rsb[:, hsl, None].broadcast_to([P, 4, D]))
                    nc.sync.dma_start(
                        y_r[T][:, 512 * half:512 * half + 512],
                        out[:, hsl, :].rearrange("p j e -> p (j e)"))
            else:
                nc.vector.tensor_mul(
                    out[:], avps[:],
                    rsb[:, :, None].broadcast_to([P, 8, D]))
                nc.sync.dma_start(
                    y_r[T], out[:].rearrange("p j e -> p (j e)"))

        LAG = int(BUFS.get("lag", 1))
        BACKFIRST = bool(BUFS.get("backfirst", 0))
        for T in range(NT):
            if BACKFIRST:
                back(T - LAG)
                front(T)
            else:
                front(T)
                back(T - LAG)
        for T in range(NT - LAG, NT):
            back(T)

    nc.compile()
    return nc


def _get_nc(mm_dt_name="f32"):
    if mm_dt_name not in _NC_CACHE:
        _NC_CACHE[mm_dt_name] = _build_nc(mm_dt_name)
    return _NC_CACHE[mm_dt_name]


def _prep_in_maps(x, Wq, bq, Wk, bk, Wv, bv, mm_dt_name="f32"):
    import ml_dtypes
    if mm_dt_name == "bf16":
        mm_np = ml_dtypes.bfloat16
    else:
        mm_np = np.float32
    s = np.float32(1.0 / np.sqrt(D))
    Wq = np.asarray(Wq, np.float64)
    Wk = np.asarray(Wk, np.float64)
    zmt = np.ascontiguousarray(s * (Wq.T @ Wk)).astype(mm_np)
    ucol = (s * (Wk.T @ np.asarray(bq, np.float64))).reshape(D, 1).astype(
        np.float32)
    wvt = np.ascontiguousarray(np.asarray(Wv).T).astype(mm_np)
    bvb = np.tile(np.asarray(bv).reshape(1, D).astype(np.float32), (D, 4))
    a = np.float32(176.0)   # a*a = 30976; exact in fp8 e4m3
    mka = np.zeros((32, D), np.float32)
    mkb = np.zeros((32, D), np.float32)
    mka[0, :] = a
    mkb[0, :] = -a
    for j in range(16):
        mka[1 + j, 8 * j:8 * j + 8] = a
        mkb[1 + j, 8 * j:8 * j + 8] = a
    mkb = np.tile(mkb, (1, 4))
    # DoubleRow layout [K=16, 2, cols]: result = sum_i A_i^T B_i
    mka = mka.reshape(2, 16, D).transpose(1, 0, 2).reshape(16, 2 * D)
    mkb = mkb.reshape(2, 16, 4 * D).transpose(1, 0, 2).reshape(16, 8 * D)
    mka = mka.astype(ml_dtypes.float8_e4m3)
    mkb = mkb.astype(ml_dtypes.float8_e4m3)
    one = np.ones((D, 1), mm_np)
    xs = np.asarray(x, np.float32).reshape(B * N, F)
    shared = dict(zmt=zmt, wvt=wvt, ucol=ucol, bvb=bvb, mka=mka,
                  mkb=mkb, one=one)
    in_maps = []
    for c in range(NCORES):
        xc = xs[c * TOK:(c + 1) * TOK]
        # xt[d, t*8+h] = x[t, h*128+d]
        xtc = np.ascontiguousarray(
            xc.reshape(TOK, HEADS, D).transpose(2, 0, 1).reshape(
                D, TOK * HEADS)).astype(mm_np)
        in_maps.append(dict(xt=xtc, **shared))
    return in_maps


def run(x, Wq, bq, Wk, bk, Wv, bv, mm_dt_name="f32", run_bufs=None,
        **run_kw):
    from concourse.bass_utils import run_bass_kernel_spmd

    nc = _build_nc(mm_dt_name, BUFS=run_bufs) if run_bufs else _get_nc(
        mm_dt_name)
    in_maps = _prep_in_maps(x, Wq, bq, Wk, bk, Wv, bv, mm_dt_name)
    res = run_bass_kernel_spmd(nc, in_maps, core_ids=list(range(NCORES)),
                               **run_kw)
    scr = bool((run_bufs or {}).get("scr_out", 1))
    yl = []
    for c in range(NCORES):
        a = np.asarray(res.results[c]["y"], np.float32)
        if scr:
            a = a.reshape(NT, 16, 8, 8, D).transpose(0, 3, 1, 2, 4).reshape(
                TOK, F)
        yl.append(a)
    y = np.concatenate(yl, axis=0).reshape(B, N, F)
    return y, res


def kernel(x, Wq, bq, Wk, bk, Wv, bv):
    y, _ = run(x, Wq, bq, Wk, bk, Wv, bv, mm_dt_name="bf16")
    return y



# revision 6
# speedup vs baseline: 1.0658x; 1.0085x over previous
"""Trainium2 Bass kernel v3 for nn_BlockSelfAttention (attention over 8 heads per token).
Baseline structure + fp8-DoubleRow mask matmul + bf16 output DMA.

Math per token t (32768 tokens total, 1024 features = 8 heads x 128 dims):
  xh = x[t].reshape(8, 128)                     # (h, d)
  q = xh @ Wq.T + bq ; k = xh @ Wk.T + bk ; v = xh @ Wv.T + bv
  scores = (q @ k.T) / sqrt(128)                # (8, 8) attention over heads
  out[t] = softmax(scores, -1) @ v              # -> reshape back to 1024

Identities used:
  * bk drops out (adds a per-row constant to scores -> softmax invariant).
  * 1/sqrt(d) and bq are folded into Wq/bq on the host.
  * bv is added to v rows; since softmax rows sum to 1 the output gets +bv.

Layout strategy (per core: 4096 tokens = 32 tiles of 128 tokens):
  * the host pre-transposes/interleaves x into xt[d, t*8+h] so each SBUF
    tile XT [d=128, 1024] holds 128 tokens with every 128-column block
    covering 16 whole tokens (all 8 heads).
  * q/k projections: matmul(lhsT=WqT [d,e], rhs=XT) -> qT2/kT2 [e, (t*8+h)].
  * v projection: matmul(lhsT=XT block [(d),(t,g)], rhs=WvT) ->
    V [(t,g), e] (natural orientation) per 16-token group.
  * scores (transposed): matmul(lhsT=kT2 block, rhs=qT2 block) ->
    S [(t,g), (t,h)] 128x128 per 16-token group; a rank-17 mask matmul
    pre-accumulates -30000 on off-token-diagonal entries (so exp -> 0).
  * exp on scalar engine (no max subtraction: |scores| < ~10).
  * denominator: matmul(lhsT=P, rhs=ones) -> [(t,h), 1], batched 8/bank.
  * AV: matmul(lhsT=P, rhs=V) -> out [(t,h), e]; multiply by reciprocal
    denominator (per-partition broadcast) and DMA out; the (t,h) x e tile
    maps to contiguous DRAM rows.
"""

import numpy as np

HEADS = 8
D = 128
B, N, F = 8, 4096, 1024
NCORES = 8
TOK = (B * N) // NCORES          # tokens per core
P = 128                          # tokens per tile
NT = TOK // P                    # tiles per core
NEG = -30000.0

_NC_CACHE = {}


def _build_nc(mm_dt_name="f32", BUFS=None, reps=1):
    import concourse.mybir as mybir
    import concourse.tile as tile
    from concourse import bacc
    from contextlib import ExitStack

    f32 = mybir.dt.float32
    bf16 = mybir.dt.bfloat16
    if mm_dt_name in ("f32", "f32r"):
        mm_dt = f32
    elif mm_dt_name == "bf16":
        mm_dt = bf16
    else:
        raise ValueError(mm_dt_name)

    def mm(ap):
        # reinterpret an fp32 AP as fp32r at matmul call sites
        if mm_dt_name == "f32r" and ap.dtype == f32:
            return ap.bitcast(mybir.dt.float32r)
        return ap

    BUFS = BUFS or {}
    SCR_OUT = bool(BUFS.get("scr_out", 1))
    nc = bacc.Bacc("TRN2", target_bir_lowering=False, debug=False)

    xt = nc.dram_tensor("xt", [D, TOK * HEADS], mm_dt, kind="ExternalInput")
    zmt = nc.dram_tensor("zmt", [D, D], mm_dt, kind="ExternalInput")
    wvt = nc.dram_tensor("wvt", [D, D], mm_dt, kind="ExternalInput")
    ucol = nc.dram_tensor("ucol", [D, 1], f32, kind="ExternalInput")
    bvb = nc.dram_tensor("bvb", [D, 512], f32, kind="ExternalInput")
    fp8 = mybir.dt.float8e4
    mka = nc.dram_tensor("mka", [16, 2 * D], fp8, kind="ExternalInput")
    mkb = nc.dram_tensor("mkb", [16, 2 * 4 * D], fp8, kind="ExternalInput")
    one = nc.dram_tensor("one", [D, 1], mm_dt, kind="ExternalInput")
    y = nc.dram_tensor("y", [TOK, F], bf16, kind="ExternalOutput")

    xt_r = xt.ap().rearrange("d (T c) -> T d c", c=P * HEADS)
    # DRAM element address for out tile T, group j, partition p=(t%16)*8+h, e:
    # (T*128 + 16j + p//8)*1024 + (p%8)*128 + e = T*131072 + j*16384 + p*128 + e
    if SCR_OUT:
        # scrambled: tile-row-major dump; host un-permutes
        y_r = y.ap().rearrange("(T p) c -> T p c", p=P)
    else:
        y_r = y.ap().flatten().rearrange(
            "(T j p e) -> T p j e", T=NT, j=8, p=P, e=D
        )

    AF = mybir.ActivationFunctionType

    with tile.TileContext(nc) as tc, ExitStack() as es:
        cpool = es.enter_context(tc.tile_pool(name="consts", bufs=1))
        zmt_s = cpool.tile([D, D], mm_dt, tag="zmt")
        wvt_s = cpool.tile([D, D], mm_dt, tag="wvt")
        ucol_s = cpool.tile([D, 1], f32, tag="ucol")
        bvb_s = cpool.tile([D, 512], f32, tag="bvb")
        mka_s = cpool.tile([16, 2, D], fp8, tag="mka")
        mkb_s = cpool.tile([16, 2, 4 * D], fp8, tag="mkb")
        one_s = cpool.tile([D, 1], mm_dt, tag="one")
        for t_, d_ in (
            (zmt_s, zmt), (wvt_s, wvt), (bvb_s, bvb),
            (ucol_s, ucol), (one_s, one),
        ):
            nc.scalar.dma_start(t_[:], d_.ap())
        nc.scalar.dma_start(
            mka_s[:].rearrange("k two d -> k (two d)"), mka.ap())
        nc.scalar.dma_start(
            mkb_s[:].rearrange("k two d -> k (two d)"), mkb.ap())
        bvb_v = bvb_s[:].rearrange("p (j e) -> p j e", e=D)

        pxt = es.enter_context(tc.tile_pool(name="pxt", bufs=BUFS.get("pxt", 3)))
        pz = es.enter_context(tc.tile_pool(name="pz", bufs=BUFS.get("pz", 2)))
        pv = es.enter_context(tc.tile_pool(name="pv", bufs=BUFS.get("pv", 4)))
        ppt = es.enter_context(tc.tile_pool(name="ppt", bufs=BUFS.get("ppt", 4)))
        pdr = es.enter_context(tc.tile_pool(name="pdr", bufs=BUFS.get("pdr", 3)))
        po = es.enter_context(tc.tile_pool(name="po", bufs=BUFS.get("po", 4)))
        ps = es.enter_context(tc.tile_pool(
            name="ps", bufs=BUFS.get("ps", 6), space="PSUM"))
        pav = es.enter_context(tc.tile_pool(
            name="pav", bufs=BUFS.get("pav", 1), space="PSUM"))
        # warm the ACT exp table while the first DMAs are in flight
        warm = cpool.tile([1, 2], f32, tag="warm")
        nc.gpsimd.memset(warm[:], 0.0)
        nc.scalar.activation(warm[0:1, 0:1], warm[0:1, 1:2], AF.Exp)
        WARM_MM = int(BUFS.get("warm_mm", 14))
        if WARM_MM:
            # spin the PE p-state up while the first input DMA is in flight
            wsb = cpool.tile([D, 512], mm_dt, tag="wsb")
            nc.gpsimd.memset(wsb[:], 0.0)
            for w in range(WARM_MM):
                wps = ps.tile([D, 512], f32, tag="ps", name=f"wps{w}")
                nc.tensor.matmul(wps[:], wsb[:, 0:D], wsb[:],
                                 start=True, stop=True)

        pts, vss = {}, {}

        def front(T):
            """DMA in + z/V projections + scores + exp for tile T."""
            XT = pxt.tile([D, P * HEADS], mm_dt, tag="xt", name=f"xt{T}")
            nc.sync.dma_start(XT[:], xt_r[T])

            # ---- z projection: zT2 = (s*Wk^T Wq) x + s*Wk^T bq ----
            # scoresT[(t,g),(t,h)] = x_g . z_h reproduces k.(q*s+bq*s)
            zT2 = pz.tile([D, P * HEADS], mm_dt, tag="z", name=f"z{T}")
            for half in range(2):
                csl = slice(512 * half, 512 * half + 512)
                zps = ps.tile([D, 512], f32, tag="ps", name=f"zps{T}_{half}")
                nc.tensor.matmul(zps[:], mm(zmt_s[:]), mm(XT[:, csl]),
                                 start=True, stop=True)
                nc.scalar.activation(zT2[:, csl], zps[:], AF.Identity,
                                     bias=ucol_s[:, 0:1])

            # ---- v projection -> V [(t,g), j, e] ----
            V = pv.tile([P, 8, D], mm_dt, tag="v", name=f"v{T}")
            vss[T] = V
            for half in range(2):
                vps = ps.tile([P, 4, D], f32, tag="ps", name=f"vps{T}_{half}")
                for jj in range(4):
                    j = 4 * half + jj
                    nc.tensor.matmul(vps[:, jj, :],
                                     mm(XT[:, 128 * j:128 * j + 128]),
                                     mm(wvt_s[:]), start=True, stop=True)
                nc.vector.tensor_add(V[:, 4 * half:4 * half + 4, :], vps[:],
                                     bvb_v[:])

            # ---- scores (transposed) + mask + exp -> PT [(t,g), j, (t,h)] ----
            PT = ppt.tile([P, 8, P], mm_dt, tag="pt", name=f"pt{T}")
            pts[T] = PT
            for half in range(2):
                sps = ps.tile([P, 4, P], f32, tag="ps", name=f"sps{T}_{half}")
                nc.tensor.matmul(sps[:], mka_s[:], mkb_s[:],
                                 start=True, stop=False,
                                 perf_mode=mybir.MatmulPerfMode.DoubleRow)
                for jj in range(4):
                    j = 4 * half + jj
                    gsl = slice(128 * j, 128 * j + 128)
                    nc.tensor.matmul(sps[:, jj, :], mm(XT[:, gsl]),
                                     mm(zT2[:, gsl]), start=False, stop=True,
                                     skip_group_check=True)
                nc.scalar.activation(PT[:, 4 * half:4 * half + 4, :], sps[:],
                                     AF.Exp)

        def back(T):
            """denominators + AV + out for tile T (runs one iter later so
            the exp -> denominator -> AV chain never stalls the PE)."""
            if T < 0 or T >= NT:
                return
            PT = pts.pop(T)
            V = vss.pop(T)
            dps = ps.tile([P, 8], f32, tag="ps", name=f"dps{T}")
            for j in range(8):
                nc.tensor.matmul(dps[:, j:j + 1], mm(PT[:, j, :]),
                                 mm(one_s[:]), start=True, stop=True)
            rsb = pdr.tile([P, 8], f32, tag="rs", name=f"rs{T}")
            nc.vector.reciprocal(rsb[:], dps[:])
            out = po.tile([P, 8, D], bf16, tag="o", name=f"o{T}")
            TAIL = T >= NT - int(BUFS.get("tailn", 1))
            if TAIL:
                avt = [ps.tile([P, 4, D], f32, tag="ps", name=f"avt{T}_{h}")
                       for h in range(2)]
                for j in range(8):
                    nc.tensor.matmul(avt[j // 4][:, j % 4, :],
                                     mm(PT[:, j, :]),
                                     mm(V[:, j, :]), start=True, stop=True)
            else:
                avps = pav.tile([P, 8, D], f32, tag="av", name=f"av{T}")
                for j in range(8):
                    nc.tensor.matmul(avps[:, j, :], mm(PT[:, j, :]),
                                     mm(V[:, j, :]), start=True, stop=True)
            if TAIL:
                # drain the tail: DMA each half as soon as it is scaled
                for half in range(2):
                    hsl = slice(4 * half, 4 * half + 4)
                    nc.vector.tensor_mul(
                        out[:, hsl, :], avt[half][:],
                        rsb[:, hsl, None].broadcast_to([P, 4, D]))
                    nc.sync.dma_start(
                        y_r[T][:, 512 * half:512 * half + 512],
                        out[:, hsl, :].rearrange("p j e -> p (j e)"))
            else:
                nc.vector.tensor_mul(
                    out[:], avps[:],
                    rsb[:, :, None].broadcast_to([P, 8, D]))
                nc.sync.dma_start(
                    y_r[T], out[:].rearrange("p j e -> p (j e)"))

        LAG = int(BUFS.get("lag", 1))
        BACKFIRST = bool(BUFS.get("backfirst", 0))
        for T in range(NT):
            if BACKFIRST:
                back(T - LAG)
                front(T)
            else:
                front(T)
                back(T - LAG)
        for T in range(NT - LAG, NT):
            back(T)

    nc.compile()
    return nc


def _get_nc(mm_dt_name="f32"):
    if mm_dt_name not in _NC_CACHE:
        _NC_CACHE[mm_dt_name] = _build_nc(mm_dt_name)
    return _NC_CACHE[mm_dt_name]


def _prep_in_maps(x, Wq, bq, Wk, bk, Wv, bv, mm_dt_name="f32"):
    import ml_dtypes
    if mm_dt_name == "bf16":
        mm_np = ml_dtypes.bfloat16
    else:
        mm_np = np.float32
    s = np.float32(1.0 / np.sqrt(D))
    Wq = np.asarray(Wq, np.float64)
    Wk = np.asarray(Wk, np.float64)
    zmt = np.ascontiguousarray(s * (Wq.T @ Wk)).astype(mm_np)
    ucol = (s * (Wk.T @ np.asarray(bq, np.float64))).reshape(D, 1).astype(
        np.float32)
    wvt = np.ascontiguousarray(np.asarray(Wv).T).astype(mm_np)
    bvb = np.tile(np.asarray(bv).reshape(1, D).astype(np.float32), (D, 4))
    a = np.float32(176.0)   # a*a = 30976; exact in fp8 e4m3
    mka = np.zeros((32, D), np.float32)
    mkb = np.zeros((32, D), np.float32)
    mka[0, :] = a
    mkb[0, :] = -a
    for j in range(16):
        mka[1 + j, 8 * j:8 * j + 8] = a
        mkb[1 + j, 8 * j:8 * j + 8] = a
    mkb = np.tile(mkb, (1, 4))
    # DoubleRow layout [K=16, 2, cols]: result = sum_i A_i^T B_i
    mka = mka.reshape(2, 16, D).transpose(1, 0, 2).reshape(16, 2 * D)
    mkb = mkb.reshape(2, 16, 4 * D).transpose(1, 0, 2).reshape(16, 8 * D)
    mka = mka.astype(ml_dtypes.float8_e4m3)
    mkb = mkb.astype(ml_dtypes.float8_e4m3)
    one = np.ones((D, 1), mm_np)
    xs = np.asarray(x, np.float32).reshape(B * N, F)
    shared = dict(zmt=zmt, wvt=wvt, ucol=ucol, bvb=bvb, mka=mka,
                  mkb=mkb, one=one)
    in_maps = []
    for c in range(NCORES):
        xc = xs[c * TOK:(c + 1) * TOK]
        # xt[d, t*8+h] = x[t, h*128+d]
        xtc = np.ascontiguousarray(
            xc.reshape(TOK, HEADS, D).transpose(2, 0, 1).reshape(
                D, TOK * HEADS)).astype(mm_np)
        in_maps.append(dict(xt=xtc, **shared))
    return in_maps


def run(x, Wq, bq, Wk, bk, Wv, bv, mm_dt_name="f32", run_bufs=None,
        **run_kw):
    from concourse.bass_utils import run_bass_kernel_spmd

    nc = _build_nc(mm_dt_name, BUFS=run_bufs) if run_bufs else _get_nc(
        mm_dt_name)
    in_maps = _prep_in_maps(x, Wq, bq, Wk, bk, Wv, bv, mm_dt_name)
    res = run_bass_kernel_spmd(nc, in_maps, core_ids=list(range(NCORES)),
                               **run_kw)
    scr = bool((run_bufs or {}).get("scr_out", 1))
    yl = []
    for c in range(NCORES):
        a = np.asarray(res.results[c]["y"], np.float32)
        if scr:
            a = a.reshape(NT, 16, 8, 8, D).transpose(0, 3, 1, 2, 4).reshape(
                TOK, F)
        yl.append(a)
    y = np.concatenate(yl, axis=0).reshape(B, N, F)
    return y, res


def kernel(x, Wq, bq, Wk, bk, Wv, bv):
    y, _ = run(x, Wq, bq, Wk, bk, Wv, bv, mm_dt_name="bf16")
    return y

